# revision 20
# baseline (speedup 1.0000x reference)
"""Trainium2 Bass kernel for GQA multi-head attention (B=4, S=2048, HID=1280,
NH=16, NKV=4, HD=80) sharded over 8 NeuronCores as (batch x kv-head-group).

Per core (b, hg): 8 q heads / 2 kv heads of batch b.
  A1: Q/K projection, d-major bf16 matmuls -> packed qkcols, then DMA-repacked
      per-head into Q_T[d, h, t], K_T[d, kv, t] (per-tb halves for overlap)
  A2: V projection, token-major -> V'[t, kc, kv*97] with a ones column at 96
      that makes the PV matmul emit the softmax denominator for free
  B:  scores (bf16) -> exp (ACT, exact causal spans) -> 0/1 mask mul on
      diagonal chunks (DVE) -> PV accumulate -> normalize (recip+bcast+mul)
  D:  o_proj row-parallel partial -> f16 out; host sums the two partials f32.

Schedule: A(qb+1) units and D(qb-1) JBS-chunks are dripped into B(qb)'s
in-order PE queue after each head, filling exp-wait bubbles.  Startup
interleaves wt/xt chunk DMAs across the SP and ACT queues so the first A1
group starts within ~2us; xtb0 is persistent so hardware-loop timing
iterations stay correct.

Notes from HW measurement (loop-delta timing): fp8 DoubleRow matmuls and
extra small matmuls lose to fixed per-matmul overheads (~45ns each; weight
loads are hidden for bf16 128-contraction streams); zero-padding the K=80
scores contraction to 128 bought nothing.  Stage decomposition (skip_b=1
build): A+D ~148us, B ~167us, total ~316us ~= PE columns (229us) +
1320 matmuls x ~45ns (60us) + ~27us startup/drain -- i.e. ~91% of the
practical PE floor for this algorithm; further gains need fewer/larger
matmuls, which PSUM bank limits (512 f32 out max) and the 128-partition
contraction cap mostly preclude.
"""

import functools
import math

import numpy as np
import ml_dtypes

import concourse.bass as bass
import concourse.mybir as mybir
import concourse.tile as tile
from concourse import bacc

B, S, HID = 4, 2048, 1280
NH, NKV, HD = 16, 4, 80
G = NH // NKV  # 4
Q_SIZE, KV_SIZE = NH * HD, NKV * HD
NCORE = 8
HL = 8          # local q heads per core
KVL = 2         # local kv heads per core
LQ = HL * HD    # 640 local q cols
LKV = KVL * HD  # 160 local k (and v) cols
NQKV = LQ + 2 * LKV  # 960 local qkv cols

F32 = mybir.dt.float32
F32R = mybir.dt.float32r
F16 = mybir.dt.float16
BF16 = mybir.dt.bfloat16
F8 = mybir.dt.float8e4
F8E5 = mybir.dt.float8e5

TB = 512        # stage-A token block
QB = 512        # stage-B q block
KC = 128        # k chunk (partitions)


def _build(s, causal, bias, rep=1, loop_n=0, skip_b=0, skip_a2=0):
    """Build + compile the per-core Bass program. Same program on all cores."""
    nqb = s // QB
    ntb = s // TB
    nkc_hid = HID // KC  # 10
    n_tc = s // 128
    tb_per_qb = QB // TB   # 2
    kc_per_qb = QB // KC   # 4

    if bias:
        raise NotImplementedError("assumes zero qkv bias")
    nc = bacc.Bacc(None)
    xt = nc.declare_dram_parameter("xt", [HID, s], BF16, isOutput=False)
    wt = nc.declare_dram_parameter("wt", [HID, NQKV], BF16, isOutput=False)
    owt = nc.declare_dram_parameter("owt", [LQ, HID], BF16, isOutput=False)
    if causal:
        m01 = nc.declare_dram_parameter("m01", [QB // KC, KC, QB], BF16,
                                        isOutput=False)
    else:
        m01 = nc.declare_dram_parameter("m01", [s // KC, KC, s], BF16,
                                        isOutput=False)
    out = nc.declare_dram_parameter("out", [s, HID], F16, isOutput=True)

    nkc_a = nkc_hid
    xt_r = xt[:].rearrange("(c p) t -> p c t", p=128)
    wt_r = wt[:].rearrange("(c p) n -> p c n", p=128)
    VW = 97  # 80 v cols + 16 zero pad + ones col at 96
    NM = (NQKV + 127) // 128  # 8 qkv m-chunks (960 = 7*128 + 64)
    ident = nc.declare_dram_parameter("ident", [128, 256], BF16, isOutput=False)

    with tile.TileContext(nc) as tc:
        with (
            tc.tile_pool(name="persist", bufs=1) as persist,
            tc.tile_pool(name="wtp", bufs=1) as wtp,
            tc.tile_pool(name="xtp", bufs=2) as xtp,
            tc.tile_pool(name="bwork", bufs=2) as bwork,
            tc.tile_pool(name="psA", bufs=2, space="PSUM") as psA,
            tc.tile_pool(name="psSC", bufs=4, space="PSUM") as psSC,
            tc.tile_pool(name="psPV", bufs=2, space="PSUM") as psPV,
        ):
            # ---- persistent SBUF, split per qb-block for fine-grained deps
            q_l = [persist.tile([80, HL, QB], BF16, name=f"q{j}")
                   for j in range(nqb)]
            k_l = [persist.tile([80, KVL, QB], BF16, name=f"k{j}")
                   for j in range(nqb)]
            v_l = [persist.tile([128, kc_per_qb, 2 * VW], BF16, name=f"v{j}")
                   for j in range(nqb)]
            o_pk = [persist.tile([128, LQ // 128, QB], BF16, name=f"opk{j}")
                    for j in range(nqb)]
            if causal:
                mask_sb = persist.tile([128, QB // KC, QB], BF16)

            qkpk_l = [None] * nqb

            # ---- stage A weights spread across 2 DMA queues, interleaved
            # with the first x block so A1 starts within ~2us
            wt_sb = wtp.tile([128, nkc_hid, NQKV], BF16)
            xtb0 = persist.tile([128, nkc_hid, TB], BF16, name="xtb0")
            for c_ in range(nkc_hid):
                nc.sync.dma_start(wt_sb[:, c_, :], wt_r[:, c_, 0:NQKV])
                nc.scalar.dma_start(xtb0[:, c_, :], xt_r[:, c_, 0:TB])
            ident_sb = wtp.tile([128, 256], BF16, name="ident_sb")
            nc.scalar.dma_start(ident_sb[:], ident[:])

            for j in range(nqb):
                nc.vector.memset(v_l[j][:], 0.0)
                nc.vector.memset(v_l[j][:, :, 96:97], 1.0)
                nc.vector.memset(v_l[j][:, :, VW + 96:VW + 97], 1.0)
                if skip_b:  # diagnostic build: D reads o_pk without B
                    nc.vector.memset(o_pk[j][:], 0.0)

            owt_sb = wtp.tile([128, LQ // 128, HID], BF16, name="owt_sb")
            # gpsimd (SWDGE) queue: keep the sync HWDGE queue free for the
            # startup-critical wt + first xt block loads
            if causal:
                nc.gpsimd.dma_start(
                    mask_sb[:], m01[:].rearrange("m p q -> p m q"))
            nc.gpsimd.dma_start(
                owt_sb[:], owt[:].rearrange("(c p) j -> p c j", p=128))

            import contextlib
            loop_cm = tc.For_i(0, loop_n, 1) if loop_n else contextlib.nullcontext()
            with loop_cm:
              for _rep in range(rep):
                # ---------------- stage A: QKV projection ----------------
                # prep_a issues the DMAs and returns drippable unit closures
                # so A-work can fill PE bubbles inside stage B.
                def prep_a(tb, xtb_pre=None):
                    jb, toff = tb // tb_per_qb, (tb % tb_per_qb) * TB
                    if xtb_pre is not None:
                        xtb = xtb_pre
                    else:
                        xtb = xtp.tile([128, nkc_hid, TB], BF16, tag="xtb")
                        for c_ in range(nkc_hid):
                            nc.sync.dma_start(
                                xtb[:, c_, :],
                                xt_r[:, c_, tb * TB:(tb + 1) * TB])

                    # A1: Q/K/V packed m-chunks of 128 qkvcols (960 = 7*128+64)
                    if tb % tb_per_qb == 0:
                        qkpk_l[jb] = bwork.tile([128, NM, QB], BF16,
                                                name="qkpk", tag="qkpk",
                                                bufs=2)
                    qk_pk = qkpk_l[jb]

                    def a1_unit(m, dve_copy):
                        mw = 128 if m < NM - 1 else NQKV - (NM - 1) * 128
                        msl = slice(m * 128, m * 128 + mw)
                        ps = psA.tile([128, TB], F32, tag="a")
                        for c in range(nkc_a):
                            nc.tensor.matmul(
                                ps[0:mw, :], wt_sb[:, c, msl], xtb[:, c, :],
                                start=(c == 0), stop=(c == nkc_a - 1),
                            )
                        if dve_copy:
                            nc.vector.tensor_copy(
                                qk_pk[0:mw, m, toff:toff + TB], ps[0:mw, :])
                        else:
                            nc.scalar.copy(
                                qk_pk[0:mw, m, toff:toff + TB], ps[0:mw, :])

                    def vt_unit(tci):
                        # PE-transpose this 128-token chunk's V cols (qkv_pk
                        # cols 800-959: chunk6 p32:128 + chunk7 p0:64) into
                        # token-major v_l [t, kv*97+d].  lhsT/rhs must share
                        # base partition, so chunk6 splits at p64 and the
                        # identity tile carries per-base eye blocks.
                        if skip_a2:
                            return
                        tc_g = tb * (TB // 128) + tci
                        tsl = slice(toff + tci * 128, toff + (tci + 1) * 128)
                        t1a = psA.tile([128, 32], BF16, tag="a")
                        nc.tensor.transpose(
                            t1a[:, :], qk_pk[32:64, 6, tsl],
                            ident_sb[32:64, 128:160])
                        t1b = psA.tile([128, 64], BF16, tag="a")
                        nc.tensor.transpose(
                            t1b[:, :], qk_pk[64:128, 6, tsl],
                            ident_sb[64:128, 160:224])
                        t2 = psA.tile([128, 64], BF16, tag="a")
                        nc.tensor.transpose(
                            t2[:, :], qk_pk[0:64, 7, tsl],
                            ident_sb[0:64, 0:64])
                        dst = v_l[tc_g // kc_per_qb][:, tc_g % kc_per_qb, :]
                        nc.vector.tensor_copy(dst[:, 0:32], t1a[:, 0:32])
                        nc.vector.tensor_copy(dst[:, 32:80], t1b[:, 0:48])
                        nc.vector.tensor_copy(dst[:, 97:113], t1b[:, 48:64])
                        nc.vector.tensor_copy(dst[:, 113:177], t2[:, 0:64])

                    def repack_unit():
                        # repack this tb's half of qkcols -> per-head
                        tsl_r = slice(toff, toff + TB)
                        for hh in range(HL + KVL):
                            col0 = hh * 80
                            p0, c0 = col0 % 128, col0 // 128
                            n0 = min(80, 128 - p0)
                            dstt = (q_l[jb][:, hh, :] if hh < HL
                                    else k_l[jb][:, hh - HL, :])
                            nc.sync.dma_start(
                                dstt[0:n0, tsl_r],
                                qk_pk[p0:p0 + n0, c0, tsl_r])
                            if n0 < 80:
                                nc.sync.dma_start(
                                    dstt[n0:80, tsl_r],
                                    qk_pk[0:80 - n0, c0 + 1, tsl_r])

                    units = []
                    for m in range(NM):
                        units.append(functools.partial(
                            a1_unit, m, (m % 2 == 0 or xtb_pre is not None)))
                    for tci in range(TB // 128):
                        units.append(functools.partial(vt_unit, tci))
                    units.append(repack_unit)
                    return units

                def emit_a(tb, xtb_pre=None):
                    for u in prep_a(tb, xtb_pre):
                        u()

                # ---------------- stage D: o_proj partial -----------------
                # one JBS chunk: 5 accumulating matmuls -> direct PSUM->HBM
                def emit_d_chunk(tci, j0, jn):
                    jb, toff = tci // kc_per_qb, (tci % kc_per_qb) * 128
                    nch = LQ // 128
                    ps = psA.tile([128, 512], F32, tag="a")
                    for c in range(nch):
                        nc.tensor.matmul(
                            ps[0:128, 0:jn],
                            o_pk[jb][:, c, toff:toff + 128],
                            owt_sb[:, c, j0:j0 + jn],
                            start=(c == 0), stop=(c == nch - 1),
                        )
                    stg = bwork.tile([128, 512], F16, tag="stg", bufs=3)
                    nc.vector.tensor_copy(stg[0:128, 0:jn], ps[0:128, 0:jn])
                    nc.sync.dma_start(
                        out[tci * 128:(tci + 1) * 128, j0:j0 + jn],
                        stg[0:128, 0:jn])

                # ---------------- stage B: attention (qb outer) ----------
                # dwork: deferred work units (A(qb+1), D(qb-1)) dripped into
                # the PE queue after each head to fill exp-wait bubbles
                def emit_b(qb, dwork=()):
                    dwork = list(dwork)
                    if skip_b:
                        for u in dwork:
                            u()
                        return
                    o_wk = bwork.tile([80, HL, QB], BF16, tag="ow", bufs=2)
                    nkc = (qb + 1) * kc_per_qb if causal else s // KC
                    ngrp = nkc // 2

                    def _qlo(kc):
                        return max(0, kc * KC - qb * QB) if causal else 0

                    for h in range(HL):
                        kv = h // G
                        pv = psPV.tile([97, QB], F32, tag="pv")
                        for kc in range(nkc):
                            ql = _qlo(kc)
                            sc = psSC.tile([128, QB], F32, tag="sc")
                            nc.tensor.matmul(
                                sc[:, ql:QB],
                                k_l[kc // kc_per_qb][
                                    :, kv,
                                    (kc % kc_per_qb) * KC:
                                    (kc % kc_per_qb + 1) * KC],
                                q_l[qb][:, h, ql:QB],
                                start=True, stop=True,
                            )
                            pt = bwork.tile([128, QB], BF16, tag="pt",
                                            bufs=8)
                            nc.scalar.activation(
                                pt[:, ql:QB], sc[:, ql:QB],
                                mybir.ActivationFunctionType.Exp)
                            if causal:
                                mi = kc - qb * kc_per_qb
                                if 0 <= mi < kc_per_qb:
                                    nc.vector.tensor_mul(
                                        pt[:, ql:QB], pt[:, ql:QB],
                                        mask_sb[:, mi, ql:QB],
                                    )
                            else:
                                mt = bwork.tile([128, QB], BF16, tag="mt",
                                                bufs=4)
                                nc.sync.dma_start(
                                    mt[:],
                                    m01[kc, :, qb * QB:(qb + 1) * QB])
                                nc.vector.tensor_mul(
                                    pt[:, :], pt[:, :], mt[:])
                            nc.tensor.matmul(
                                pv[0:97, ql:QB],
                                v_l[kc // kc_per_qb][
                                    :, kc % kc_per_qb,
                                    kv * VW:(kv + 1) * VW],
                                pt[:, ql:QB],
                                start=(kc == 0), stop=(kc == nkc - 1),
                                skip_group_check=True,
                            )
                        # normalize: o = pv[0:80] * (1/pv[96])
                        den = bwork.tile([1, QB], F32, tag="den")
                        nc.vector.tensor_copy(den[:], pv[96:97, :])
                        r_sb = bwork.tile([1, QB], F32, tag="r")
                        nc.vector.reciprocal_approx_fast(r_sb[:], den[:])
                        r_bc = bwork.tile([80, QB], F32, tag="rbc")
                        nc.gpsimd.partition_broadcast(r_bc[:], r_sb[:])
                        nc.vector.tensor_mul(
                            o_wk[:, h, :], pv[0:80, :], r_bc[:])
                        # pack this head into o_pk (hd = h*80+d linear)
                        hd0 = h * 80
                        p0, c0 = hd0 % 128, hd0 // 128
                        n0 = min(80, 128 - p0)
                        nc.gpsimd.dma_start(
                            o_pk[qb][p0:p0 + n0, c0, :], o_wk[0:n0, h, :])
                        if n0 < 80:
                            nc.gpsimd.dma_start(
                                o_pk[qb][0:80 - n0, c0 + 1, :],
                                o_wk[n0:80, h, :])
                        # drip deferred work units into the PE queue
                        take = (len(dwork) + HL - 1 - h) // (HL - h)
                        for _ in range(take):
                            dwork.pop(0)()
                    for u in dwork:
                        u()

                JBS = [(0, 512), (512, 512), (1024, 256)]

                def d_chunks(jb):
                    return [functools.partial(emit_d_chunk, tci, j0, jn)
                            for tci in range(4 * jb, 4 * jb + 4)
                            for (j0, jn) in JBS]

                if causal:
                    # software-pipelined: A(qb+1) units and D(qb-1) chunks
                    # drip into B(qb)'s PE queue to fill exp-wait bubbles
                    for t_ in range(tb_per_qb):
                        emit_a(t_, xtb0 if t_ == 0 else None)
                    for qb in range(nqb):
                        drip = []
                        if qb > 0:
                            drip += d_chunks(qb - 1)
                        if qb + 1 < nqb:
                            for t_ in range((qb + 1) * tb_per_qb,
                                            (qb + 2) * tb_per_qb):
                                drip += prep_a(t_)
                        emit_b(qb, drip)
                    for u in d_chunks(nqb - 1):
                        u()
                else:
                    for tb in range(ntb):
                        emit_a(tb, xtb0 if tb == 0 else None)
                    for qb in range(nqb):
                        emit_b(qb)
                    for tci in range(n_tc):
                        for (j0, jn) in JBS:
                            emit_d_chunk(tci, j0, jn)
    nc.compile()
    return nc


# ---------------------------------------------------------------------------
# cached PJRT runner (replica of bass2jax.run_bass_via_pjrt with jit reuse)
# ---------------------------------------------------------------------------
@functools.lru_cache(maxsize=8)
def _get_runner(s, causal, bias, rep=1, loop_n=0, skip_b=0, skip_a2=0):
    import jax
    import jax.numpy as jnp
    from jax.sharding import Mesh, PartitionSpec
    from jax.experimental.shard_map import shard_map
    from concourse import bass2jax
    from concourse import mybir as _mybir

    nc = _build(s, causal, bias, rep, loop_n, skip_b, skip_a2)
    bass2jax.install_neuronx_cc_hook()

    partition_name = (
        nc.partition_id_tensor.name if nc.partition_id_tensor else None
    )
    in_names, out_names, out_avals, zero_outs = [], [], [], []
    for alloc in nc.m.functions[0].allocations:
        if not isinstance(alloc, _mybir.MemoryLocationSet):
            continue
        name = alloc.memorylocations[0].name
        if alloc.kind == "ExternalInput":
            if name != partition_name:
                in_names.append(name)
        elif alloc.kind == "ExternalOutput":
            shape = tuple(alloc.tensor_shape)
            dtype = _mybir.dt.np(alloc.dtype)
            out_names.append(name)
            out_avals.append(jax.core.ShapedArray(shape, dtype))
            zero_outs.append(np.zeros(shape, dtype))
    n_params = len(in_names)
    n_outs = len(out_avals)
    all_names = in_names + out_names
    if partition_name is not None:
        all_names = all_names + [partition_name]

    def _body(*args):
        operands = list(args)
        if partition_name is not None:
            operands.append(bass2jax.partition_id_tensor())
        outs = bass2jax._bass_exec_p.bind(
            *operands,
            out_avals=tuple(out_avals),
            in_names=tuple(all_names),
            out_names=tuple(out_names),
            lowering_input_output_aliases=(),
            sim_require_finite=True,
            sim_require_nnan=True,
            nc=nc,
        )
        return tuple(outs)

    devices = jax.devices()[:NCORE]
    mesh = Mesh(np.asarray(devices), ("core",))
    donate = tuple(range(n_params, n_params + n_outs))
    sharded = jax.jit(
        shard_map(
            _body, mesh=mesh,
            in_specs=(PartitionSpec("core"),) * (n_params + n_outs),
            out_specs=(PartitionSpec("core"),) * n_outs,
            check_rep=False,
        ),
        donate_argnums=donate,
        keep_unused=True,
    )

    def run(in_maps):
        from jax.sharding import NamedSharding
        sh = NamedSharding(mesh, PartitionSpec("core"))
        concat_in = [
            np.concatenate([np.asarray(m[name]) for m in in_maps], axis=0)
            for name in in_names
        ]
        concat_zeros = [
            jnp.zeros((NCORE * z.shape[0], *z.shape[1:]), z.dtype, device=sh)
            for z in zero_outs
        ]
        out_arrs = sharded(*concat_in, *concat_zeros)
        return [
            {
                name: np.asarray(out_arrs[i]).reshape(
                    NCORE, *out_avals[i].shape)[c]
                for i, name in enumerate(out_names)
            }
            for c in range(NCORE)
        ]

    def bench(in_maps, iters=10):
        """Time device execution with device-resident inputs, no donation."""
        from jax.sharding import NamedSharding
        import time as _time

        nodonate = jax.jit(
            shard_map(
                _body, mesh=mesh,
                in_specs=(PartitionSpec("core"),) * (n_params + n_outs),
                out_specs=(PartitionSpec("core"),) * n_outs,
                check_rep=False,
            ),
            keep_unused=True,
        )
        sh = NamedSharding(mesh, PartitionSpec("core"))
        dev_in = [
            jax.device_put(
                np.concatenate([np.asarray(m[name]) for m in in_maps], axis=0),
                sh)
            for name in in_names
        ]
        dev_zeros = [
            jax.device_put(
                np.zeros((NCORE * z.shape[0], *z.shape[1:]), z.dtype), sh)
            for z in zero_outs
        ]
        out = nodonate(*dev_in, *dev_zeros)
        jax.block_until_ready(out)
        times = []
        for _ in range(iters):
            t0 = _time.perf_counter()
            out = nodonate(*dev_in, *dev_zeros)
            jax.block_until_ready(out)
            times.append(_time.perf_counter() - t0)
        return times

    def bench_chain(in_maps, chain, iters=5):
        """Chain `chain` kernel executions in one dispatch (output buffer of
        call i feeds call i+1 as the to-be-overwritten out buffer), so the
        ~70ms axon dispatch overhead amortizes. Returns list of wall times."""
        from jax.sharding import NamedSharding
        import time as _time

        assert n_outs == 1

        def _chained(*args):
            ins, out = list(args[:n_params]), args[n_params]
            for _ in range(chain):
                out = _body(*ins, out)[0]
            return out

        f = jax.jit(
            shard_map(
                _chained, mesh=mesh,
                in_specs=(PartitionSpec("core"),) * (n_params + 1),
                out_specs=PartitionSpec("core"),
                check_rep=False,
            ),
            keep_unused=True,
        )
        sh = NamedSharding(mesh, PartitionSpec("core"))
        dev_in = [
            jax.device_put(
                np.concatenate([np.asarray(m[name]) for m in in_maps], axis=0),
                sh)
            for name in in_names
        ]
        z = zero_outs[0]
        dev_zero = jax.device_put(
            np.zeros((NCORE * z.shape[0], *z.shape[1:]), z.dtype), sh)
        jax.block_until_ready(f(*dev_in, dev_zero))
        times = []
        for _ in range(iters):
            t0 = _time.perf_counter()
            jax.block_until_ready(f(*dev_in, dev_zero))
            times.append(_time.perf_counter() - t0)
        return times

    run.bench = bench
    run.bench_chain = bench_chain
    return run


# ---------------------------------------------------------------------------
# host wrapper
# ---------------------------------------------------------------------------
def _softplus(x):
    return np.logaddexp(0.0, x).astype(np.float32)


def _causal_mask_tiles():
    kk = np.arange(KC)[:, None]
    qq = np.arange(QB)[None, :]
    tiles = np.stack(
        [(qq >= kk + m * KC) for m in range(QB // KC)]
    ).astype(ml_dtypes.bfloat16)
    return tiles  # [4, 128, 512]


def _is_causal(mask, neg=-2.3819763e38):
    m = mask.reshape(mask.shape[-2], mask.shape[-1])
    expect = np.where(
        np.tril(np.ones(m.shape, dtype=bool)), np.float32(0.0), np.float32(neg)
    )
    return np.array_equal(m, expect)


def prepare_inputs(hidden_states, mask, scaling, qkv_w, qkv_b, o_w, o_b):
    s = hidden_states.shape[1]
    hidden_states = np.asarray(hidden_states, dtype=np.float32)
    mask = np.asarray(mask, dtype=np.float32)
    scaling = np.asarray(scaling, dtype=np.float32)
    qkv_w = np.asarray(qkv_w, dtype=np.float32)
    qkv_b = np.asarray(qkv_b, dtype=np.float32)
    o_w = np.asarray(o_w, dtype=np.float32)
    o_b = np.asarray(o_b, dtype=np.float32)

    causal = bool(_is_causal(mask))
    bias = bool(np.any(qkv_b))

    scale = (1.442695041 / math.sqrt(HD)) * _softplus(scaling)  # [80]
    wq = qkv_w[:Q_SIZE] * np.tile(scale, NH)[:, None]           # scaled

    if causal:
        m01_full = _causal_mask_tiles()
    else:
        # exp(mask) transposed to [k, q], tiled as [s/128, 128, s]
        me = np.exp(mask.reshape(s, s).T.astype(np.float32))
        m01_full = np.ascontiguousarray(
            me.reshape(s // KC, KC, s)).astype(ml_dtypes.bfloat16)

    # xt depends only on batch; wt/owt only on head-group -> build each once
    xts = []
    hs_bf = hidden_states.astype(ml_dtypes.bfloat16)
    for b in range(B):
        xts.append(np.ascontiguousarray(hs_bf[b].T))           # [1280, s]
    wts, owts = [], []
    for hg in range(2):
        qrows = slice(hg * LQ, (hg + 1) * LQ)
        krows = slice(Q_SIZE + hg * LKV, Q_SIZE + (hg + 1) * LKV)
        vrows = slice(Q_SIZE + KV_SIZE + hg * LKV,
                      Q_SIZE + KV_SIZE + (hg + 1) * LKV)
        w_slice = np.concatenate(
            [wq[qrows], qkv_w[krows], qkv_w[vrows]], axis=0)   # [960, 1280]
        wts.append(np.ascontiguousarray(
            w_slice.T).astype(ml_dtypes.bfloat16))             # [1280, 960]
        owts.append(np.ascontiguousarray(
            o_w[:, hg * LQ:(hg + 1) * LQ].T).astype(ml_dtypes.bfloat16))
    # per-base-partition identity blocks for PE transpose:
    # cols 0:128 = eye(128) (base-0), cols 128:160 = eye(32) at rows 32:64
    # (base-32), cols 160:224 = eye(64) at rows 64:128 (base-64)
    ident_np = np.zeros((128, 256), dtype=ml_dtypes.bfloat16)
    ident_np[0:128, 0:128] = np.eye(128, dtype=ml_dtypes.bfloat16)
    ident_np[32:64, 128:160] = np.eye(32, dtype=ml_dtypes.bfloat16)
    ident_np[64:128, 160:224] = np.eye(64, dtype=ml_dtypes.bfloat16)
    in_maps = []
    for c in range(NCORE):
        b, hg = divmod(c, 2)
        in_maps.append({"xt": xts[b], "wt": wts[hg],
                        "owt": owts[hg], "m01": m01_full,
                        "ident": ident_np})
    return in_maps, causal, bias, o_b


def kernel(hidden_states, mask, scaling, qkv_w, qkv_b, o_w, o_b):
    s = hidden_states.shape[1]
    in_maps, causal, bias, o_b32 = prepare_inputs(
        hidden_states, mask, scaling, qkv_w, qkv_b, o_w, o_b)
    run = _get_runner(s, causal, bias)
    res = run(in_maps)
    out = np.empty((B, s, HID), dtype=np.float32)
    for b in range(B):
        out[b] = (res[2 * b]["out"].astype(np.float32)
                  + res[2 * b + 1]["out"].astype(np.float32)
                  + o_b32[None, :])
    return out



# revision 27
# speedup vs baseline: 1.0487x; 1.0487x over previous
"""Trainium2 Bass kernel for GQA multi-head attention (B=4, S=2048, HID=1280,
NH=16, NKV=4, HD=80) sharded over 8 NeuronCores as (batch x kv-head-group).

Per core (b, hg): 8 q heads / 2 kv heads of batch b.
  A1: Q/K projection, d-major bf16 matmuls -> packed qkcols, then DMA-repacked
      per-head into Q_T[d, h, t], K_T[d, kv, t] (per-tb halves for overlap)
  A2: V projection, token-major -> V'[t, kc, kv*97] with a ones column at 96
      that makes the PV matmul emit the softmax denominator for free
  B:  scores (bf16) -> exp (ACT, exact causal spans) -> 0/1 mask mul on
      diagonal chunks (DVE) -> PV accumulate -> normalize (recip+bcast+mul)
  D:  o_proj row-parallel partial -> f16 out; host sums the two partials f32.

Schedule: A(qb+1) units and D(qb-1) JBS-chunks are dripped into B(qb)'s
in-order PE queue after each head, filling exp-wait bubbles.  Startup
interleaves wt/xt chunk DMAs across the SP and ACT queues so the first A1
group starts within ~2us; xtb0 is persistent so hardware-loop timing
iterations stay correct.

Notes from HW measurement (loop-delta timing): fp8 DoubleRow matmuls and
extra small matmuls lose to fixed per-matmul overheads (~45ns each; weight
loads are hidden for bf16 128-contraction streams); zero-padding the K=80
scores contraction to 128 bought nothing.  Stage decomposition (skip_b=1
build): A+D ~148us, B ~167us, total ~316us ~= PE columns (229us) +
1320 matmuls x ~45ns (60us) + ~27us startup/drain -- i.e. ~91% of the
practical PE floor for this algorithm; further gains need fewer/larger
matmuls, which PSUM bank limits (512 f32 out max) and the 128-partition
contraction cap mostly preclude.
"""

import functools
import math

import numpy as np
import ml_dtypes

import concourse.bass as bass
import concourse.mybir as mybir
import concourse.tile as tile
from concourse import bacc

B, S, HID = 4, 2048, 1280
NH, NKV, HD = 16, 4, 80
G = NH // NKV  # 4
Q_SIZE, KV_SIZE = NH * HD, NKV * HD
NCORE = 8
HL = 8          # local q heads per core
KVL = 2         # local kv heads per core
LQ = HL * HD    # 640 local q cols
LKV = KVL * HD  # 160 local k (and v) cols
NQKV = LQ + 2 * LKV  # 960 local qkv cols

F32 = mybir.dt.float32
F32R = mybir.dt.float32r
F16 = mybir.dt.float16
BF16 = mybir.dt.bfloat16
F8 = mybir.dt.float8e4
F8E5 = mybir.dt.float8e5

TB = 512        # stage-A token block
QB = 512        # stage-B q block
KC = 128        # k chunk (partitions)


def _build(s, causal, bias, rep=1, loop_n=0, skip_b=0, skip_a2=0):
    """Build + compile the per-core Bass program. Same program on all cores."""
    nqb = s // QB
    ntb = s // TB
    nkc_hid = HID // KC  # 10
    n_tc = s // 128
    tb_per_qb = QB // TB   # 2
    kc_per_qb = QB // KC   # 4

    if bias:
        raise NotImplementedError("assumes zero qkv bias")
    nc = bacc.Bacc(None)
    xt = nc.declare_dram_parameter("xt", [HID, s], BF16, isOutput=False)
    wt = nc.declare_dram_parameter("wt", [HID, NQKV], BF16, isOutput=False)
    owt = nc.declare_dram_parameter("owt", [LQ, HID], BF16, isOutput=False)
    if causal:
        m01 = nc.declare_dram_parameter("m01", [QB // KC, KC, QB], BF16,
                                        isOutput=False)
    else:
        m01 = nc.declare_dram_parameter("m01", [s // KC, KC, s], BF16,
                                        isOutput=False)
    out = nc.declare_dram_parameter("out", [s, HID], F16, isOutput=True)

    nkc_a = nkc_hid
    xt_r = xt[:].rearrange("(c p) t -> p c t", p=128)
    wt_r = wt[:].rearrange("(c p) n -> p c n", p=128)
    VW = 97  # 80 v cols + 16 zero pad + ones col at 96


    with tile.TileContext(nc) as tc:
        with (
            tc.tile_pool(name="persist", bufs=1) as persist,
            tc.tile_pool(name="wtp", bufs=1) as wtp,
            tc.tile_pool(name="xtp", bufs=2) as xtp,
            tc.tile_pool(name="bwork", bufs=2) as bwork,
            tc.tile_pool(name="psA", bufs=2, space="PSUM") as psA,
            tc.tile_pool(name="psSC", bufs=4, space="PSUM") as psSC,
            tc.tile_pool(name="psPV", bufs=2, space="PSUM") as psPV,
        ):
            # ---- persistent SBUF, split per qb-block for fine-grained deps
            q_l = [persist.tile([80, HL, QB], BF16, name=f"q{j}")
                   for j in range(nqb)]
            k_l = [persist.tile([80, KVL, QB], BF16, name=f"k{j}")
                   for j in range(nqb)]
            v_l = [persist.tile([128, kc_per_qb, 2 * VW], BF16, name=f"v{j}")
                   for j in range(nqb)]
            o_pk = [persist.tile([128, LQ // 128, QB], BF16, name=f"opk{j}")
                    for j in range(nqb)]
            if causal:
                mask_sb = persist.tile([128, QB // KC, QB], BF16)

            qkpk_l = [None] * nqb

            # ---- stage A weights spread across 2 DMA queues, interleaved
            # with the first x block so A1 starts within ~2us
            wt_sb = wtp.tile([128, nkc_hid, LQ + LKV], BF16)
            xtb0 = persist.tile([128, nkc_hid, TB], BF16, name="xtb0")
            for c_ in range(nkc_hid):
                nc.sync.dma_start(wt_sb[:, c_, :], wt_r[:, c_, 0:LQ + LKV])
                nc.scalar.dma_start(xtb0[:, c_, :], xt_r[:, c_, 0:TB])
            wtv_bf = wtp.tile([128, nkc_hid, LKV], BF16)
            nc.scalar.dma_start(wtv_bf[:], wt_r[:, :, LQ + LKV:NQKV])

            for j in range(nqb):
                nc.vector.memset(v_l[j][:], 0.0)
                nc.vector.memset(v_l[j][:, :, 96:97], 1.0)
                nc.vector.memset(v_l[j][:, :, VW + 96:VW + 97], 1.0)
                if skip_b:  # diagnostic build: D reads o_pk without B
                    nc.vector.memset(o_pk[j][:], 0.0)

            owt_sb = wtp.tile([128, LQ // 128, HID], BF16, name="owt_sb")
            # gpsimd (SWDGE) queue: keep the sync HWDGE queue free for the
            # startup-critical wt + first xt block loads
            if causal:
                nc.gpsimd.dma_start(
                    mask_sb[:], m01[:].rearrange("m p q -> p m q"))
            nc.gpsimd.dma_start(
                owt_sb[:], owt[:].rearrange("(c p) j -> p c j", p=128))

            import contextlib
            loop_cm = tc.For_i(0, loop_n, 1) if loop_n else contextlib.nullcontext()
            with loop_cm:
              for _rep in range(rep):
                # ---------------- stage A: QKV projection ----------------
                # prep_a issues the DMAs and returns drippable unit closures
                # so A-work can fill PE bubbles inside stage B.
                def prep_a(tb, xtb_pre=None):
                    jb, toff = tb // tb_per_qb, (tb % tb_per_qb) * TB
                    if xtb_pre is not None:
                        xtb = xtb_pre
                    else:
                        xtb = xtp.tile([128, nkc_hid, TB], BF16, tag="xtb")
                        for c_ in range(nkc_hid):
                            nc.sync.dma_start(
                                xtb[:, c_, :],
                                xt_r[:, c_, tb * TB:(tb + 1) * TB])

                    # A1: Q & K packed m-chunks of 128 qkcols (800 = 6*128+32)
                    if tb % tb_per_qb == 0:
                        qkpk_l[jb] = bwork.tile([128, 7, QB], BF16,
                                                name="qkpk", tag="qkpk",
                                                bufs=2)
                    qk_pk = qkpk_l[jb]

                    def a1_unit(m, dve_copy):
                        mw = 128 if m < 6 else 32
                        msl = slice(m * 128, m * 128 + mw)
                        ps = psA.tile([128, TB], F32, tag="a")
                        for c in range(nkc_a):
                            nc.tensor.matmul(
                                ps[0:mw, :], wt_sb[:, c, msl], xtb[:, c, :],
                                start=(c == 0), stop=(c == nkc_a - 1),
                            )
                        if dve_copy:
                            nc.vector.tensor_copy(
                                qk_pk[0:mw, m, toff:toff + TB], ps[0:mw, :])
                        else:
                            nc.scalar.copy(
                                qk_pk[0:mw, m, toff:toff + TB], ps[0:mw, :])

                    def a2_unit(tci):
                        if skip_a2:
                            return
                        tc_g = tb * (TB // 128) + tci
                        tsl = slice(tci * 128, (tci + 1) * 128)
                        ps = psA.tile([128, LKV], F32, tag="a")
                        for c in range(nkc_a):
                            nc.tensor.matmul(
                                ps[:], xtb[:, c, tsl], wtv_bf[:, c, :],
                                start=(c == 0), stop=(c == nkc_a - 1),
                            )
                        dst = v_l[tc_g // kc_per_qb][
                            :, tc_g % kc_per_qb, :].rearrange(
                            "p (kv e) -> p kv e", kv=2)[:, :, 0:HD]  # e = VW
                        src = ps[:].rearrange("p (kv e) -> p kv e", kv=2)
                        nc.vector.tensor_copy(dst, src)

                    def repack_unit():
                        # repack this tb's half of qkcols -> per-head
                        tsl_r = slice(toff, toff + TB)
                        for hh in range(HL + KVL):
                            col0 = hh * 80
                            p0, c0 = col0 % 128, col0 // 128
                            n0 = min(80, 128 - p0)
                            dstt = (q_l[jb][:, hh, :] if hh < HL
                                    else k_l[jb][:, hh - HL, :])
                            nc.sync.dma_start(
                                dstt[0:n0, tsl_r],
                                qk_pk[p0:p0 + n0, c0, tsl_r])
                            if n0 < 80:
                                nc.sync.dma_start(
                                    dstt[n0:80, tsl_r],
                                    qk_pk[0:80 - n0, c0 + 1, tsl_r])

                    units = []
                    for m in range(7):
                        units.append(functools.partial(
                            a1_unit, m, (m % 2 == 0 or xtb_pre is not None)))
                    for tci in range(TB // 128):
                        units.append(functools.partial(a2_unit, tci))
                    units.append(repack_unit)
                    return units

                def emit_a(tb, xtb_pre=None):
                    for u in prep_a(tb, xtb_pre):
                        u()

                # ---------------- stage D: o_proj partial -----------------
                # one JBS chunk: 5 accumulating matmuls -> direct PSUM->HBM
                def emit_d_chunk(tci, j0, jn):
                    jb, toff = tci // kc_per_qb, (tci % kc_per_qb) * 128
                    nch = LQ // 128
                    ps = psA.tile([128, 512], F32, tag="a")
                    for c in range(nch):
                        nc.tensor.matmul(
                            ps[0:128, 0:jn],
                            o_pk[jb][:, c, toff:toff + 128],
                            owt_sb[:, c, j0:j0 + jn],
                            start=(c == 0), stop=(c == nch - 1),
                        )
                    stg = bwork.tile([128, 512], F16, tag="stg", bufs=3)
                    nc.vector.tensor_copy(stg[0:128, 0:jn], ps[0:128, 0:jn])
                    nc.sync.dma_start(
                        out[tci * 128:(tci + 1) * 128, j0:j0 + jn],
                        stg[0:128, 0:jn])

                # ---------------- stage B: attention (qb outer) ----------
                # dwork: deferred work units (A(qb+1), D(qb-1)) dripped into
                # the PE queue after each head to fill exp-wait bubbles
                def emit_b(qb, dwork=()):
                    dwork = list(dwork)
                    if skip_b:
                        for u in dwork:
                            u()
                        return
                    o_wk = bwork.tile([80, HL, QB], BF16, tag="ow", bufs=2)
                    nkc = (qb + 1) * kc_per_qb if causal else s // KC
                    ngrp = nkc // 2

                    def _qlo(kc):
                        return max(0, kc * KC - qb * QB) if causal else 0

                    def _ksl(kc, kv):
                        return k_l[kc // kc_per_qb][
                            :, kv,
                            (kc % kc_per_qb) * KC:(kc % kc_per_qb + 1) * KC]

                    for h in range(HL):
                        kv = h // G
                        hh = h % G
                        if causal and hh == 0:
                            # GQA merge: the two smallest diagonal chunks
                            # share stationary K across the 4 heads of this
                            # kv group -> batch their heads into one (span
                            # 128) / two (span 256) score matmuls + exps.
                            kc3 = qb * kc_per_qb + 3
                            ql3 = 3 * KC
                            sc4 = psSC.tile([128, G, KC], F32, tag="sc")
                            nc.tensor.matmul(
                                sc4[:, :, :], _ksl(kc3, kv),
                                q_l[qb][:, h:h + G, ql3:QB],
                                start=True, stop=True,
                            )
                            pt4 = bwork.tile([128, G, KC], BF16, tag="pt4",
                                             bufs=2)
                            nc.scalar.activation(
                                pt4[:], sc4[:],
                                mybir.ActivationFunctionType.Exp)
                            for i in range(G):
                                nc.vector.tensor_mul(
                                    pt4[:, i, :], pt4[:, i, :],
                                    mask_sb[:, 3, ql3:QB])
                            kc2 = qb * kc_per_qb + 2
                            ql2 = 2 * KC
                            pt2 = []
                            for pr in range(2):
                                sc2 = psSC.tile([128, 2, 2 * KC], F32,
                                                tag="sc")
                                nc.tensor.matmul(
                                    sc2[:, :, :], _ksl(kc2, kv),
                                    q_l[qb][:, h + 2 * pr:h + 2 * pr + 2,
                                            ql2:QB],
                                    start=True, stop=True,
                                )
                                p2 = bwork.tile([128, 2, 2 * KC], BF16,
                                                tag="pt2", bufs=4)
                                nc.scalar.activation(
                                    p2[:], sc2[:],
                                    mybir.ActivationFunctionType.Exp)
                                for i in range(2):
                                    nc.vector.tensor_mul(
                                        p2[:, i, :], p2[:, i, :],
                                        mask_sb[:, 2, ql2:QB])
                                pt2.append(p2)
                        pv = psPV.tile([97, QB], F32, tag="pv")
                        for kc in range(nkc):
                            ql = _qlo(kc)
                            mi = kc - qb * kc_per_qb
                            if causal and mi == 3:
                                ptk = pt4[:, hh, :]
                            elif causal and mi == 2:
                                ptk = pt2[hh // 2][:, hh % 2, :]
                            else:
                                sc = psSC.tile([128, QB], F32, tag="sc")
                                nc.tensor.matmul(
                                    sc[:, ql:QB], _ksl(kc, kv),
                                    q_l[qb][:, h, ql:QB],
                                    start=True, stop=True,
                                )
                                pt = bwork.tile([128, QB], BF16, tag="pt",
                                                bufs=8)
                                nc.scalar.activation(
                                    pt[:, ql:QB], sc[:, ql:QB],
                                    mybir.ActivationFunctionType.Exp)
                                if causal:
                                    if 0 <= mi < kc_per_qb:
                                        nc.vector.tensor_mul(
                                            pt[:, ql:QB], pt[:, ql:QB],
                                            mask_sb[:, mi, ql:QB],
                                        )
                                else:
                                    mt = bwork.tile([128, QB], BF16,
                                                    tag="mt", bufs=4)
                                    nc.sync.dma_start(
                                        mt[:],
                                        m01[kc, :, qb * QB:(qb + 1) * QB])
                                    nc.vector.tensor_mul(
                                        pt[:, :], pt[:, :], mt[:])
                                ptk = pt[:, ql:QB]
                            nc.tensor.matmul(
                                pv[0:97, ql:QB],
                                v_l[kc // kc_per_qb][
                                    :, kc % kc_per_qb,
                                    kv * VW:(kv + 1) * VW],
                                ptk,
                                start=(kc == 0), stop=(kc == nkc - 1),
                                skip_group_check=True,
                            )
                        # normalize: o = pv[0:80] * (1/pv[96])
                        den = bwork.tile([1, QB], F32, tag="den")
                        nc.vector.tensor_copy(den[:], pv[96:97, :])
                        r_sb = bwork.tile([1, QB], F32, tag="r")
                        nc.vector.reciprocal_approx_fast(r_sb[:], den[:])
                        r_bc = bwork.tile([80, QB], F32, tag="rbc")
                        nc.gpsimd.partition_broadcast(r_bc[:], r_sb[:])
                        nc.vector.tensor_mul(
                            o_wk[:, h, :], pv[0:80, :], r_bc[:])
                        # pack this head into o_pk (hd = h*80+d linear)
                        hd0 = h * 80
                        p0, c0 = hd0 % 128, hd0 // 128
                        n0 = min(80, 128 - p0)
                        nc.gpsimd.dma_start(
                            o_pk[qb][p0:p0 + n0, c0, :], o_wk[0:n0, h, :])
                        if n0 < 80:
                            nc.gpsimd.dma_start(
                                o_pk[qb][0:80 - n0, c0 + 1, :],
                                o_wk[n0:80, h, :])
                        # drip deferred work units into the PE queue
                        take = (len(dwork) + HL - 1 - h) // (HL - h)
                        for _ in range(take):
                            dwork.pop(0)()
                    for u in dwork:
                        u()

                JBS = [(0, 512), (512, 512), (1024, 256)]

                def d_chunks(jb):
                    return [functools.partial(emit_d_chunk, tci, j0, jn)
                            for tci in range(4 * jb, 4 * jb + 4)
                            for (j0, jn) in JBS]

                if causal:
                    # software-pipelined: A(qb+1) units and D(qb-1) chunks
                    # drip into B(qb)'s PE queue to fill exp-wait bubbles
                    for t_ in range(tb_per_qb):
                        emit_a(t_, xtb0 if t_ == 0 else None)
                    for qb in range(nqb):
                        drip = []
                        if qb > 0:
                            drip += d_chunks(qb - 1)
                        if qb + 1 < nqb:
                            for t_ in range((qb + 1) * tb_per_qb,
                                            (qb + 2) * tb_per_qb):
                                drip += prep_a(t_)
                        emit_b(qb, drip)
                    for u in d_chunks(nqb - 1):
                        u()
                else:
                    for tb in range(ntb):
                        emit_a(tb, xtb0 if tb == 0 else None)
                    for qb in range(nqb):
                        emit_b(qb)
                    for tci in range(n_tc):
                        for (j0, jn) in JBS:
                            emit_d_chunk(tci, j0, jn)
    nc.compile()
    return nc


# ---------------------------------------------------------------------------
# cached PJRT runner (replica of bass2jax.run_bass_via_pjrt with jit reuse)
# ---------------------------------------------------------------------------
@functools.lru_cache(maxsize=8)
def _get_runner(s, causal, bias, rep=1, loop_n=0, skip_b=0, skip_a2=0):
    import jax
    import jax.numpy as jnp
    from jax.sharding import Mesh, PartitionSpec
    from jax.experimental.shard_map import shard_map
    from concourse import bass2jax
    from concourse import mybir as _mybir

    nc = _build(s, causal, bias, rep, loop_n, skip_b, skip_a2)
    bass2jax.install_neuronx_cc_hook()

    partition_name = (
        nc.partition_id_tensor.name if nc.partition_id_tensor else None
    )
    in_names, out_names, out_avals, zero_outs = [], [], [], []
    for alloc in nc.m.functions[0].allocations:
        if not isinstance(alloc, _mybir.MemoryLocationSet):
            continue
        name = alloc.memorylocations[0].name
        if alloc.kind == "ExternalInput":
            if name != partition_name:
                in_names.append(name)
        elif alloc.kind == "ExternalOutput":
            shape = tuple(alloc.tensor_shape)
            dtype = _mybir.dt.np(alloc.dtype)
            out_names.append(name)
            out_avals.append(jax.core.ShapedArray(shape, dtype))
            zero_outs.append(np.zeros(shape, dtype))
    n_params = len(in_names)
    n_outs = len(out_avals)
    all_names = in_names + out_names
    if partition_name is not None:
        all_names = all_names + [partition_name]

    def _body(*args):
        operands = list(args)
        if partition_name is not None:
            operands.append(bass2jax.partition_id_tensor())
        outs = bass2jax._bass_exec_p.bind(
            *operands,
            out_avals=tuple(out_avals),
            in_names=tuple(all_names),
            out_names=tuple(out_names),
            lowering_input_output_aliases=(),
            sim_require_finite=True,
            sim_require_nnan=True,
            nc=nc,
        )
        return tuple(outs)

    devices = jax.devices()[:NCORE]
    mesh = Mesh(np.asarray(devices), ("core",))
    donate = tuple(range(n_params, n_params + n_outs))
    sharded = jax.jit(
        shard_map(
            _body, mesh=mesh,
            in_specs=(PartitionSpec("core"),) * (n_params + n_outs),
            out_specs=(PartitionSpec("core"),) * n_outs,
            check_rep=False,
        ),
        donate_argnums=donate,
        keep_unused=True,
    )

    def run(in_maps):
        from jax.sharding import NamedSharding
        sh = NamedSharding(mesh, PartitionSpec("core"))
        concat_in = [
            np.concatenate([np.asarray(m[name]) for m in in_maps], axis=0)
            for name in in_names
        ]
        concat_zeros = [
            jnp.zeros((NCORE * z.shape[0], *z.shape[1:]), z.dtype, device=sh)
            for z in zero_outs
        ]
        out_arrs = sharded(*concat_in, *concat_zeros)
        return [
            {
                name: np.asarray(out_arrs[i]).reshape(
                    NCORE, *out_avals[i].shape)[c]
                for i, name in enumerate(out_names)
            }
            for c in range(NCORE)
        ]

    def bench(in_maps, iters=10):
        """Time device execution with device-resident inputs, no donation."""
        from jax.sharding import NamedSharding
        import time as _time

        nodonate = jax.jit(
            shard_map(
                _body, mesh=mesh,
                in_specs=(PartitionSpec("core"),) * (n_params + n_outs),
                out_specs=(PartitionSpec("core"),) * n_outs,
                check_rep=False,
            ),
            keep_unused=True,
        )
        sh = NamedSharding(mesh, PartitionSpec("core"))
        dev_in = [
            jax.device_put(
                np.concatenate([np.asarray(m[name]) for m in in_maps], axis=0),
                sh)
            for name in in_names
        ]
        dev_zeros = [
            jax.device_put(
                np.zeros((NCORE * z.shape[0], *z.shape[1:]), z.dtype), sh)
            for z in zero_outs
        ]
        out = nodonate(*dev_in, *dev_zeros)
        jax.block_until_ready(out)
        times = []
        for _ in range(iters):
            t0 = _time.perf_counter()
            out = nodonate(*dev_in, *dev_zeros)
            jax.block_until_ready(out)
            times.append(_time.perf_counter() - t0)
        return times

    def bench_chain(in_maps, chain, iters=5):
        """Chain `chain` kernel executions in one dispatch (output buffer of
        call i feeds call i+1 as the to-be-overwritten out buffer), so the
        ~70ms axon dispatch overhead amortizes. Returns list of wall times."""
        from jax.sharding import NamedSharding
        import time as _time

        assert n_outs == 1

        def _chained(*args):
            ins, out = list(args[:n_params]), args[n_params]
            for _ in range(chain):
                out = _body(*ins, out)[0]
            return out

        f = jax.jit(
            shard_map(
                _chained, mesh=mesh,
                in_specs=(PartitionSpec("core"),) * (n_params + 1),
                out_specs=PartitionSpec("core"),
                check_rep=False,
            ),
            keep_unused=True,
        )
        sh = NamedSharding(mesh, PartitionSpec("core"))
        dev_in = [
            jax.device_put(
                np.concatenate([np.asarray(m[name]) for m in in_maps], axis=0),
                sh)
            for name in in_names
        ]
        z = zero_outs[0]
        dev_zero = jax.device_put(
            np.zeros((NCORE * z.shape[0], *z.shape[1:]), z.dtype), sh)
        jax.block_until_ready(f(*dev_in, dev_zero))
        times = []
        for _ in range(iters):
            t0 = _time.perf_counter()
            jax.block_until_ready(f(*dev_in, dev_zero))
            times.append(_time.perf_counter() - t0)
        return times

    run.bench = bench
    run.bench_chain = bench_chain
    return run


# ---------------------------------------------------------------------------
# host wrapper
# ---------------------------------------------------------------------------
def _softplus(x):
    return np.logaddexp(0.0, x).astype(np.float32)


def _causal_mask_tiles():
    kk = np.arange(KC)[:, None]
    qq = np.arange(QB)[None, :]
    tiles = np.stack(
        [(qq >= kk + m * KC) for m in range(QB // KC)]
    ).astype(ml_dtypes.bfloat16)
    return tiles  # [4, 128, 512]


def _is_causal(mask, neg=-2.3819763e38):
    m = mask.reshape(mask.shape[-2], mask.shape[-1])
    expect = np.where(
        np.tril(np.ones(m.shape, dtype=bool)), np.float32(0.0), np.float32(neg)
    )
    return np.array_equal(m, expect)


def prepare_inputs(hidden_states, mask, scaling, qkv_w, qkv_b, o_w, o_b):
    s = hidden_states.shape[1]
    hidden_states = np.asarray(hidden_states, dtype=np.float32)
    mask = np.asarray(mask, dtype=np.float32)
    scaling = np.asarray(scaling, dtype=np.float32)
    qkv_w = np.asarray(qkv_w, dtype=np.float32)
    qkv_b = np.asarray(qkv_b, dtype=np.float32)
    o_w = np.asarray(o_w, dtype=np.float32)
    o_b = np.asarray(o_b, dtype=np.float32)

    causal = bool(_is_causal(mask))
    bias = bool(np.any(qkv_b))

    scale = (1.442695041 / math.sqrt(HD)) * _softplus(scaling)  # [80]
    wq = qkv_w[:Q_SIZE] * np.tile(scale, NH)[:, None]           # scaled

    if causal:
        m01_full = _causal_mask_tiles()
    else:
        # exp(mask) transposed to [k, q], tiled as [s/128, 128, s]
        me = np.exp(mask.reshape(s, s).T.astype(np.float32))
        m01_full = np.ascontiguousarray(
            me.reshape(s // KC, KC, s)).astype(ml_dtypes.bfloat16)

    # xt depends only on batch; wt/owt only on head-group -> build each once
    xts = []
    hs_bf = hidden_states.astype(ml_dtypes.bfloat16)
    for b in range(B):
        xts.append(np.ascontiguousarray(hs_bf[b].T))           # [1280, s]
    wts, owts = [], []
    for hg in range(2):
        qrows = slice(hg * LQ, (hg + 1) * LQ)
        krows = slice(Q_SIZE + hg * LKV, Q_SIZE + (hg + 1) * LKV)
        vrows = slice(Q_SIZE + KV_SIZE + hg * LKV,
                      Q_SIZE + KV_SIZE + (hg + 1) * LKV)
        w_slice = np.concatenate(
            [wq[qrows], qkv_w[krows], qkv_w[vrows]], axis=0)   # [960, 1280]
        wts.append(np.ascontiguousarray(
            w_slice.T).astype(ml_dtypes.bfloat16))             # [1280, 960]
        owts.append(np.ascontiguousarray(
            o_w[:, hg * LQ:(hg + 1) * LQ].T).astype(ml_dtypes.bfloat16))
    in_maps = []
    for c in range(NCORE):
        b, hg = divmod(c, 2)
        in_maps.append({"xt": xts[b], "wt": wts[hg],
                        "owt": owts[hg], "m01": m01_full})
    return in_maps, causal, bias, o_b


def kernel(hidden_states, mask, scaling, qkv_w, qkv_b, o_w, o_b):
    s = hidden_states.shape[1]
    in_maps, causal, bias, o_b32 = prepare_inputs(
        hidden_states, mask, scaling, qkv_w, qkv_b, o_w, o_b)
    run = _get_runner(s, causal, bias)
    res = run(in_maps)
    out = np.empty((B, s, HID), dtype=np.float32)
    for b in range(B):
        out[b] = (res[2 * b]["out"].astype(np.float32)
                  + res[2 * b + 1]["out"].astype(np.float32)
                  + o_b32[None, :])
    return out

